# revision 52
# baseline (speedup 1.0000x reference)
"""Trainium2 Bass kernel for a Longformer encoder layer.

Reference computation (B=2, S=4096, DM=768, H=12, HD=64, FF=3072, w=64):
    q,k,v = split_heads(x @ Wq + bq), ...
    attn  = sliding_window_attention(q, k, v, w=64)   # |key - query| <= 64
    x1    = LN1(attn + x)
    out   = LN2(gelu(x1 @ W1 + b1) @ W2 + b2 + x1)

Distribution: sequence-parallel over 8 cores. Flat token space [B*S = 8192]
is split into 8 contiguous shards of 1024 tokens (4 shards per batch
element). Each core receives its shard plus a 64-token halo on each side
(zero-padded at batch boundaries), computes K/V for the halo'd range and
Q for its own 1024 tokens, runs attention + FFN locally. No collectives.

On-chip algorithm per core:
  1. QKV (f32r matmuls): qT [128,1024] / kT [128,1152] bf16 feature-major
     (6 tiles each); V token-major bf16 with a ones column per head
     ([128, 12*65] per 128-token block, 9 blocks).
  2. Attention, query-stationary: per query tile t (128 queries) the band
     keys are exactly halo key blocks {t, t+1}. Per half (6 heads):
     12 score matmuls -> one PSUM tile [128, 6, 256]; ONE exp over all
     1536 columns (ACT); band triangles zeroed in-place by two gpsimd
     affine_selects (no mask tensors); sequence-edge fixup via tiny DVE
     multiplies only on edge tiles; 12 PV matmuls accumulate over the two
     key blocks into PSUM [128, 6, 65] where column 64 is the softmax
     denominator; one batched reciprocal; evac+normalize fused via
     ACT/DVE ops with per-partition scale.
  3. Residual + LN1 token-major: bn_stats/bn_aggr; rstd = exp(-0.5*ln(
     var+eps)) so the ACT table set stays natural_log_exp (shared with
     the attention exp -- no table thrash); apply via one ACT op with
     per-partition scale/bias. ln1_g/ln1_b are folded into W1/b1 on the
     host. x1T (feature-major bf16) via 6 PE transposes per tile.
  4. FFN: h = gelu_apprx_tanh(W1'^T-slices @ x1T + b1') -- native gelu,
     ONE ACT op per [128,512] PSUM tile; y = hs^T @ W2 accumulated over
     24 k-tiles; z = y + x1 (DVE, also evacuates); LN2 stats per tile;
     rstd/apply batched at the end (one table switch back from the gelu
     set); DMA out.

kernel(**inputs) takes the full unsharded inputs and returns the full
[2, 4096, 768] output.
"""

import numpy as np
import ml_dtypes

B, S, DM, H, FF, WIN, HD = 2, 4096, 768, 12, 3072, 64, 64
NCORES = 8
TC = 1024          # core tokens per shard
TH = TC + 2 * WIN  # halo'd tokens = 1152
NT = TC // 128     # 8 query tiles per core
NB = TH // 128     # 9 key blocks
DK = DM // 128     # 6 feature tiles
MFF = FF // 128    # 24 ff tiles
HDE = HD + 1       # head dim + ones column

_PROGS = {}
_FLAGS = None


def _split_multi_waits(nc, mybir, max_waits=1):
    """walrus codegen accepts at most one sync-wait per instruction; hoist
    extra waits onto standalone EventSemaphore instructions."""
    n_split = 0
    for f in nc.m.functions:
        for blk in f.blocks:
            out = []
            for inst in blk.instructions:
                si = inst.sync_info
                if si is not None and si.on_wait and len(si.on_wait) > max_waits:
                    waits = list(si.on_wait)
                    for j, w in enumerate(waits[:-max_waits]):
                        ev = mybir.InstEventSemaphore(
                            name=f"{inst.name}_hw{j}", ins=[], outs=[])
                        ev.engine = inst.engine
                        ev.sync_info = mybir.SyncInfo(on_wait=[w], on_update=[])
                        out.append(ev)
                        n_split += 1
                    inst.sync_info = mybir.SyncInfo(
                        on_wait=waits[-max_waits:], on_update=list(si.on_update))
                out.append(inst)
            blk.instructions = out
    return n_split


def _build_program(flags):
    import os
    stub = os.environ.get("KSTUB", "")
    f_bv, f_b2, f_ln1, f_ln2 = flags
    import concourse.bass as bass
    import concourse.tile as tile
    from concourse import mybir
    from concourse.masks import make_identity

    f32 = mybir.dt.float32
    f32r = mybir.dt.float32r
    bf16 = mybir.dt.bfloat16
    AF = mybir.ActivationFunctionType
    OP = mybir.AluOpType

    nc = bass.Bass(target_bir_lowering=False)

    xT_h = nc.declare_dram_parameter("xT", [DM, TH], bf16, isOutput=False)
    xres_h = nc.declare_dram_parameter("xres", [TC, DM], f32, isOutput=False)
    Wq_h = nc.declare_dram_parameter("Wq", [DM, DM], bf16, isOutput=False)  # pre-scaled 1/8
    Wk_h = nc.declare_dram_parameter("Wk", [DM, DM], bf16, isOutput=False)
    Wv_h = nc.declare_dram_parameter("Wv", [DM, DM], bf16, isOutput=False)
    bq_h = nc.declare_dram_parameter("bq", [128, DK], f32, isOutput=False)  # pre-scaled 1/8
    bk_h = nc.declare_dram_parameter("bk", [128, DK], f32, isOutput=False)
    W1_h = nc.declare_dram_parameter("W1", [DM, FF], bf16, isOutput=False)  # g1-folded
    W2_h = nc.declare_dram_parameter("W2", [FF, DM], bf16, isOutput=False)
    b1_h = nc.declare_dram_parameter("b1", [128, MFF], f32, isOutput=False)  # ln1_b-folded
    # combined band+edge masks, one variant per tile class:
    # 0 = interior, 1 = first tile (left seq edge), 2 = last tile (right edge)
    mk_h = nc.declare_dram_parameter("mk", [128, 3, 6, 256], bf16, isOutput=False)
    # bf16 output halves the out-DMA; the host unshard upconverts. Keep f32
    # when a general ln2 path post-processes in place.
    out_dt = bf16 if f_ln2 else f32
    out_h = nc.declare_dram_parameter("out", [TC, DM], out_dt, isOutput=True)
    if not f_bv:
        bv_h = nc.declare_dram_parameter("bv", [H * HDE], bf16, isOutput=False)
    if not f_b2:
        b2_h = nc.declare_dram_parameter("b2", [DM], f32, isOutput=False)
    if not f_ln1:
        g1_h = nc.declare_dram_parameter("ln1g", [DM], f32, isOutput=False)
        be1_h = nc.declare_dram_parameter("ln1b", [DM], f32, isOutput=False)
    if not f_ln2:
        g2_h = nc.declare_dram_parameter("ln2g", [DM], f32, isOutput=False)
        be2_h = nc.declare_dram_parameter("ln2b", [DM], f32, isOutput=False)

    def bcast_dram(ap1d, parts=128):
        a = ap1d.ap() if hasattr(ap1d, "ap") and not isinstance(ap1d, bass.AP) else ap1d
        return bass.AP(tensor=a.tensor, offset=a.offset, ap=[[0, parts]] + list(a.ap))

    with tile.TileContext(nc) as tc:
        with tc.tile_pool(name="const", bufs=1) as pc:
            ident = pc.tile([128, 128], f32, name="ident", tag="ident")
            make_identity(nc, ident)
            eps_t = pc.tile([128, 1], f32, name="eps_t", tag="eps")
            nc.vector.memset(eps_t, 1e-5)
            bq_t = pc.tile([128, DK], f32, name="bq_t", tag="bq")
            nc.sync.dma_start(out=bq_t, in_=bq_h[:, :])
            bk_t = pc.tile([128, DK], f32, name="bk_t", tag="bk")
            nc.sync.dma_start(out=bk_t, in_=bk_h[:, :])
            b1_t = pc.tile([128, MFF], f32, name="b1_t", tag="b1")
            nc.sync.dma_start(out=b1_t, in_=b1_h[:, :])
            mk_t = pc.tile([128, 3, 6, 256], bf16, name="mk_t", tag="mk")
            if not f_bv:
                bv_t = pc.tile([128, H * HDE], bf16, name="bv_t", tag="bv")
                nc.sync.dma_start(out=bv_t, in_=bcast_dram(bv_h))
            if not f_b2:
                b2_t = pc.tile([128, DM], f32, name="b2_t", tag="b2")
                nc.sync.dma_start(out=b2_t, in_=bcast_dram(b2_h))
            if not f_ln1:
                g1_t = pc.tile([128, DM], f32, name="g1_t", tag="g1")
                nc.sync.dma_start(out=g1_t, in_=bcast_dram(g1_h))
                be1_t = pc.tile([128, DM], f32, name="be1_t", tag="be1")
                nc.sync.dma_start(out=be1_t, in_=bcast_dram(be1_h))
            if not f_ln2:
                g2_t = pc.tile([128, DM], f32, name="g2_t", tag="g2")
                nc.sync.dma_start(out=g2_t, in_=bcast_dram(g2_h))
                be2_t = pc.tile([128, DM], f32, name="be2_t", tag="be2")
                nc.sync.dma_start(out=be2_t, in_=bcast_dram(be2_h))

            with (
                tc.tile_pool(name="x1_persist", bufs=1) as pD,
                tc.tile_pool(name="wff1", bufs=1) as pW1,
            ):
                x1h = [pD.tile([128, DM], f32, name=f"x1h{t}", tag=f"x1h{t}")
                       for t in range(NT)]
                x1T = [pD.tile([128, TC], bf16, name=f"x1T{k}", tag=f"x1T{k}")
                       for k in range(DK)]
                if not f_ln1:
                    x1r = [pD.tile([128, DM], f32, name=f"x1r{t}", tag=f"x1r{t}")
                           for t in range(NT)]
                else:
                    x1r = x1h
                mv2 = pD.tile([128, NT, 2], f32, name="mv2", tag="mv2")

                with tc.tile_pool(name="qkv_persist", bufs=1) as pA:
                    qT = [pA.tile([128, TC], bf16, name=f"qT{k}", tag=f"qT{k}")
                          for k in range(DK)]
                    kT = [pA.tile([128, TH], bf16, name=f"kT{k}", tag=f"kT{k}")
                          for k in range(DK)]
                    Vx = [pA.tile([128, H * HDE], bf16, name=f"Vx{t}", tag=f"Vx{t}")
                          for t in range(NB)]

                    # ---------------- Phase 1: QKV projections ----------------
                    # -- Phases 1-3 fused: QKV, then V-projection interleaved
                    # with attention+LN1 (attention tile t runs as soon as V
                    # blocks t and t+1 exist; its exp/mask/normalize work
                    # overlaps the remaining V matmuls and keeps PE dense) --
                    with (
                        tc.tile_pool(name="ph1x", bufs=1) as p1x,
                        tc.tile_pool(name="ph1w", bufs=8) as p1w,
                        tc.tile_pool(name="ps1", bufs=2, space="PSUM") as ps1,
                        tc.tile_pool(name="ph2", bufs=3) as p2,
                        tc.tile_pool(name="ph3", bufs=2) as p3,
                        tc.tile_pool(name="ps_sc", bufs=2, space="PSUM") as psc,
                        tc.tile_pool(name="ps_pv", bufs=2, space="PSUM") as ppv,
                    ):
                        # interleave x and Wq transfers so the first q matmul
                        # (needs only xTs[0]+ws[0]) starts ~2 transfers in
                        xTs = []
                        ws = []
                        for k in range(DK):
                            t_ = p1x.tile([128, TH], bf16, name=f"xTs{k}", tag=f"xTs{k}")
                            nc.sync.dma_start(out=t_, in_=xT_h[k * 128:(k + 1) * 128, :])
                            xTs.append(t_)
                            w = p1w.tile([128, DM], bf16, name="w_rot", tag="w_rot")
                            nc.sync.dma_start(out=w, in_=Wq_h[k * 128:(k + 1) * 128, :])
                            ws.append(w)
                        for mt in range(DK):
                            for nch in range(2):
                                ps = ps1.tile([128, 512], f32, name="ps_q", tag="ps1")
                                for k in range(DK):
                                    nc.tensor.matmul(
                                        ps,
                                        lhsT=ws[k][:, mt * 128:(mt + 1) * 128],
                                        rhs=xTs[k][:, WIN + nch * 512: WIN + (nch + 1) * 512],
                                        start=(k == 0), stop=(k == DK - 1))
                                nc.vector.tensor_scalar_add(
                                    out=qT[mt][:, nch * 512:(nch + 1) * 512],
                                    in0=ps, scalar1=bq_t[:, mt:mt + 1])
                        # kT: full halo range (N=384)
                        ws = []
                        for k in range(DK):
                            w = p1w.tile([128, DM], bf16, name="w_rot", tag="w_rot")
                            nc.sync.dma_start(out=w, in_=Wk_h[k * 128:(k + 1) * 128, :])
                            ws.append(w)
                        for mt in range(DK):
                            for nch in range(3):
                                ps = ps1.tile([128, 384], f32, name="ps_k", tag="ps1")
                                for k in range(DK):
                                    nc.tensor.matmul(
                                        ps,
                                        lhsT=ws[k][:, mt * 128:(mt + 1) * 128],
                                        rhs=xTs[k][:, nch * 384:(nch + 1) * 384],
                                        start=(k == 0), stop=(k == DK - 1))
                                nc.vector.tensor_scalar_add(
                                    out=kT[mt][:, nch * 384:(nch + 1) * 384],
                                    in0=ps, scalar1=bk_t[:, mt:mt + 1])
                        # masks + V weights + residual-x + W1 transfers queued
                        # now so they ride the DMA queue behind the q/k
                        # inputs, landing during the q/k matmuls
                        nc.sync.dma_start(out=mk_t, in_=mk_h[:, :, :, :])
                        ws = []
                        for k in range(DK):
                            w = p1w.tile([128, DM], bf16, name="w_rot", tag="w_rot")
                            nc.sync.dma_start(out=w, in_=Wv_h[k * 128:(k + 1) * 128, :])
                            ws.append(w)
                        xrs = []
                        for t in range(NT):
                            xr = p3.tile([128, DM], f32, name="xr", tag="xr", bufs=8)
                            nc.sync.dma_start(out=xr,
                                              in_=xres_h[t * 128:(t + 1) * 128, :])
                            xrs.append(xr)
                        W1s = []
                        for k in range(DK):
                            t_ = pW1.tile([128, FF], bf16, name=f"W1s{k}", tag=f"W1s{k}")
                            nc.sync.dma_start(out=t_, in_=W1_h[k * 128:(k + 1) * 128, :])
                            W1s.append(t_)

                        def v_proj(tt):
                            vx3 = Vx[tt].rearrange("p (h e) -> p h e", h=H)
                            nc.vector.memset(vx3[:, :, HD:HDE], 1.0)
                            for ch in range(2):
                                ps = ps1.tile([128, 384], f32, name="ps_v", tag="ps1")
                                for k in range(DK):
                                    nc.tensor.matmul(
                                        ps,
                                        lhsT=xTs[k][:, tt * 128:(tt + 1) * 128],
                                        rhs=ws[k][:, ch * 384:(ch + 1) * 384],
                                        start=(k == 0), stop=(k == DK - 1))
                                nc.scalar.copy(
                                    out=vx3[:, ch * 6:(ch + 1) * 6, 0:HD],
                                    in_=ps.rearrange("p (h e) -> p h e", e=HD))
                            if not f_bv:
                                nc.vector.tensor_add(Vx[tt], Vx[tt], bv_t)

                        def attention(t):
                            att = p3.tile([128, DM], f32, name="att", tag="att",
                                          bufs=4)
                            mvi = (0 if 0 < t < NT - 1 else (1 if t == 0 else 2))
                            for half in range(2):
                                pv = ppv.tile([128, 6, HDE], f32, name="pv", tag="pv")
                                # 3-head score tiles (2 PSUM banks each) so
                                # everything double-buffers within 8 banks.
                                # Head parity per half: all score matmuls in
                                # one PSUM tile share one base partition
                                # (mixing PE row groups within a bank is
                                # fatal on HW).
                                for sub in range(2):
                                    sc = psc.tile([128, 3, 256], f32, name="sc",
                                                  tag="sc",
                                                  padded_shape=[128, 4, 256])
                                    for jj in range(3):
                                        h = 2 * (3 * sub + jj) + half
                                        dk, po = h // 2, (h % 2) * HD
                                        for kb in range(2):
                                            nc.tensor.matmul(
                                                sc[:, jj, kb * 128:(kb + 1) * 128],
                                                lhsT=kT[dk][po:po + HD,
                                                            (t + kb) * 128:(t + kb + 1) * 128],
                                                rhs=qT[dk][po:po + HD,
                                                           t * 128:(t + 1) * 128],
                                                start=True, stop=True)
                                    ex = p2.tile([128, 3, 256], bf16, name="ex",
                                                 tag="ex", bufs=6)
                                    nc.scalar.activation(out=ex, in_=sc, func=AF.Exp)
                                    # band triangles + sequence-edge clip in
                                    # one multiply
                                    nc.vector.tensor_mul(ex, ex, mk_t[:, mvi, 0:3])
                                    for jj in range(3):
                                        j = 3 * sub + jj
                                        h = 2 * j + half
                                        for kb in range(2):
                                            nc.tensor.matmul(
                                                pv[:, j, :],
                                                lhsT=ex[:, jj, kb * 128:(kb + 1) * 128],
                                                rhs=Vx[t + kb].rearrange(
                                                    "p (h e) -> p h e", h=H)[:, h, :],
                                                start=(kb == 0), stop=(kb == 1))
                                rc = p2.tile([128, 6, 1], f32, name="rc", tag="rc")
                                nc.vector.reciprocal(out=rc, in_=pv[:, :, HD:HDE])
                                # evac+normalize all 6 heads in one op: the
                                # reciprocal broadcasts along the head dim via
                                # a zero-stride read; the output lands on this
                                # half's interleaved head columns (stride 128)
                                rcv = bass.AP(tensor=rc.tensor, offset=rc.offset,
                                              ap=[list(rc.ap[0]), [1, 6], [0, HD]])
                                attv = att.rearrange("p (j two e) -> p j two e",
                                                     two=2, e=HD)[:, :, half, :]
                                nc.vector.tensor_tensor(out=attv, in0=pv[:, :, 0:HD],
                                                        in1=rcv, op=OP.mult)
                            nc.vector.tensor_add(att, att, xrs[t])
                            # LN1: stats + rstd(ln/exp keeps natural_log_exp set)
                            st = p3.tile([128, 3, 6], f32, name="ln_st", tag="ln_st")
                            for sg in range(3):
                                nc.vector.bn_stats(out=st[:, sg, :],
                                                   in_=att[:, sg * 256:(sg + 1) * 256])
                            mv = p3.tile([128, 2], f32, name="ln_mv", tag="ln_mv")
                            nc.vector.bn_aggr(out=mv, in_=st)
                            lv = p3.tile([128, 1], f32, name="ln_lv", tag="ln_lv")
                            nc.scalar.activation(out=lv, in_=mv[:, 1:2], func=AF.Ln,
                                                 bias=eps_t, scale=1.0)
                            rstd = p3.tile([128, 1], f32, name="ln_rstd", tag="ln_rstd")
                            nc.scalar.activation(out=rstd, in_=lv, func=AF.Exp,
                                                 scale=-0.5)
                            nmr = p3.tile([128, 1], f32, name="ln_nmr", tag="ln_nmr")
                            nc.vector.tensor_scalar(
                                out=nmr, in0=mv[:, 0:1], scalar1=rstd, scalar2=-1.0,
                                op0=OP.mult, op1=OP.mult)
                            nc.scalar.activation(out=x1h[t], in_=att, func=AF.Identity,
                                                 bias=nmr, scale=rstd)
                            if not f_ln1:
                                # true x1 = x_hat*g1 + b1 for the residual path
                                nc.vector.scalar_tensor_tensor(
                                    out=x1r[t], in0=x1h[t], scalar=1.0, in1=g1_t,
                                    op0=OP.mult, op1=OP.mult)
                                nc.vector.tensor_add(x1r[t], x1r[t], be1_t)

                        for tt in range(NB):
                            v_proj(tt)
                            if stub != "nop23" and tt >= 1:
                                attention(tt - 1)
                        if stub == "nop23":
                            for t in range(NT):
                                nc.vector.memset(x1h[t], 0.5)
                            for d in range(DK):
                                nc.vector.memset(x1T[d], 0.5)

                # ------------------- Phase 4: FFN -------------------
                with (
                    tc.tile_pool(name="wff2", bufs=1) as pW2,
                    tc.tile_pool(name="ph4h", bufs=1) as p4h,
                    tc.tile_pool(name="ph4z", bufs=1) as p4z,
                    tc.tile_pool(name="ph4t", bufs=2) as p4t,
                    tc.tile_pool(name="ps4h", bufs=2, space="PSUM") as ps4h,
                    tc.tile_pool(name="ps4y", bufs=2, space="PSUM") as ps4y,
                    tc.tile_pool(name="ps_tr", bufs=2, space="PSUM") as ptr,
                ):
                    W2s = []
                    for m in range(MFF):
                        t_ = pW2.tile([128, DM], bf16, name=f"W2s{m}", tag=f"W2s{m}")
                        nc.sync.dma_start(out=t_, in_=W2_h[m * 128:(m + 1) * 128, :])
                        W2s.append(t_)
                    # x1T transposes (feature-major bf16 for the h matmuls);
                    # PSUM banks for these fit this phase's budget
                    for t in (range(NT) if stub != "nop23" else []):
                        for d in range(DK):
                            pT = ptr.tile([128, 128], f32, name="pT", tag="pT")
                            nc.tensor.transpose(
                                out=pT, in_=x1h[t][:, d * 128:(d + 1) * 128],
                                identity=ident)
                            if d % 2 == 0:
                                nc.scalar.copy(
                                    out=x1T[d][:, t * 128:(t + 1) * 128], in_=pT)
                            else:
                                nc.vector.tensor_copy(
                                    out=x1T[d][:, t * 128:(t + 1) * 128], in_=pT)
                    hs = [p4h.tile([128, 512], bf16, name=f"hs{m}", tag=f"hs{m}")
                          for m in range(MFF)]
                    zs = [p4z.tile([128, DM], f32, name=f"z{t}", tag=f"z{t}")
                          for t in range(NT)]

                    if stub == "nop4":
                        for t in range(NT):
                            ot = p4t.tile([128, DM], out_dt, name="ot", tag="ot")
                            nc.vector.tensor_copy(out=ot, in_=x1h[t])
                            nc.sync.dma_start(
                                out=out_h[t * 128:(t + 1) * 128, :], in_=ot)
                    for half in (range(2) if stub != "nop4" else []):
                        for m in range(MFF):
                            ph = ps4h.tile([128, 512], f32, name="ph", tag="ph")
                            for k in range(DK):
                                nc.tensor.matmul(
                                    ph,
                                    lhsT=W1s[k][:, m * 128:(m + 1) * 128],
                                    rhs=x1T[k][:, half * 512:(half + 1) * 512],
                                    start=(k == 0), stop=(k == DK - 1))
                            nc.scalar.activation(
                                out=hs[m], in_=ph, func=AF.Gelu_apprx_tanh,
                                bias=b1_t[:, m:m + 1], scale=1.0)
                        for tt in range(4):
                            t = half * 4 + tt
                            py = ps4y.tile([128, 2, 384], f32, name="py", tag="py",
                                           padded_shape=[128, 2, 512])
                            for m in range(MFF):
                                for nh in range(2):
                                    nc.tensor.matmul(
                                        py[:, nh, :],
                                        lhsT=hs[m][:, tt * 128:(tt + 1) * 128],
                                        rhs=W2s[m][:, nh * 384:(nh + 1) * 384],
                                        start=(m == 0), stop=(m == MFF - 1))
                            z3 = zs[t].rearrange("p (n f) -> p n f", n=2)
                            nc.vector.tensor_add(
                                z3, py, x1r[t].rearrange("p (n f) -> p n f", n=2))
                            if not f_b2:
                                nc.vector.tensor_add(zs[t], zs[t], b2_t)
                            st2 = p4t.tile([128, 3, 6], f32, name="st2", tag="st2")
                            for sg in range(3):
                                nc.vector.bn_stats(
                                    out=st2[:, sg, :],
                                    in_=zs[t][:, sg * 256:(sg + 1) * 256])
                            nc.vector.bn_aggr(out=mv2[:, t, :], in_=st2)

                    # batched LN2 tail: one ln/exp pair for all 8 tiles (a
                    # mid-FFN batch gets reordered into the gelu stream and
                    # thrashes the ACT tables -- measured); applies split
                    # across ACT and DVE to drain in parallel
                    if stub != "nop4":
                        lv2 = p4t.tile([128, NT], f32, name="lv2", tag="lv2",
                                       bufs=1)
                        nc.scalar.activation(out=lv2, in_=mv2[:, :, 1], func=AF.Ln,
                                             bias=eps_t, scale=1.0)
                        rstd2 = p4t.tile([128, NT], f32, name="rstd2", tag="rstd2",
                                         bufs=1)
                        nc.scalar.activation(out=rstd2, in_=lv2, func=AF.Exp,
                                             scale=-0.5)
                        nmr2 = p4t.tile([128, NT], f32, name="nmr2", tag="nmr2",
                                        bufs=1)
                        nc.vector.tensor_tensor(out=nmr2, in0=mv2[:, :, 0],
                                                in1=rstd2, op=OP.mult)
                        nc.vector.tensor_scalar_mul(out=nmr2, in0=nmr2,
                                                    scalar1=-1.0)
                        for t in range(NT):
                            ot = p4t.tile([128, DM], out_dt, name="ot", tag="ot",
                                          bufs=4)
                            if t % 2 == 0:
                                nc.scalar.activation(
                                    out=ot, in_=zs[t], func=AF.Identity,
                                    bias=nmr2[:, t:t + 1], scale=rstd2[:, t:t + 1])
                            else:
                                nc.vector.tensor_scalar(
                                    out=ot, in0=zs[t], scalar1=rstd2[:, t:t + 1],
                                    scalar2=nmr2[:, t:t + 1], op0=OP.mult,
                                    op1=OP.add)
                            if not f_ln2:
                                nc.vector.tensor_mul(ot, ot, g2_t)
                                nc.vector.tensor_add(ot, ot, be2_t)
                            nc.sync.dma_start(
                                out=out_h[t * 128:(t + 1) * 128, :], in_=ot)
    return nc


def _get_program(flags=None):
    global _PROGS
    if flags is None:
        flags = _FLAGS if _FLAGS is not None else (True, True, True, True)
    key = tuple(flags)
    if key not in _PROGS:
        _PROGS[key] = {"nc": _build_program(key), "split": False}
    return _PROGS[key]["nc"]


def make_in_maps(x, Wq, bq, Wk, bk, Wv, bv, ln1_g, ln1_b, W1, b1, W2, b2,
                 ln2_g, ln2_b):
    global _FLAGS
    xf = np.asarray(x, np.float32)
    Wq_f = np.asarray(Wq, np.float32)
    bq_f = np.asarray(bq, np.float32)
    bk_f = np.asarray(bk, np.float32)
    bv_f = np.asarray(bv, np.float32)
    g1 = np.asarray(ln1_g, np.float32)
    be1 = np.asarray(ln1_b, np.float32)
    g2 = np.asarray(ln2_g, np.float32)
    be2 = np.asarray(ln2_b, np.float32)
    W1_f = np.asarray(W1, np.float32)
    b1_f = np.asarray(b1, np.float32)
    b2_f = np.asarray(b2, np.float32)

    f_bv = bool(np.all(bv_f == 0.0))
    f_b2 = bool(np.all(b2_f == 0.0))
    f_ln1 = bool(np.all(g1 == 1.0) and np.all(be1 == 0.0))
    f_ln2 = bool(np.all(g2 == 1.0) and np.all(be2 == 0.0))
    _FLAGS = (f_bv, f_b2, f_ln1, f_ln2)

    Wq_s = np.ascontiguousarray(
        (Wq_f * (1.0 / np.sqrt(HD))).astype(ml_dtypes.bfloat16))
    bq_s = np.ascontiguousarray(
        (bq_f * (1.0 / np.sqrt(HD))).reshape(DK, 128).T)
    bk_r = np.ascontiguousarray(bk_f.reshape(DK, 128).T)
    # fold ln1_g into W1 rows and ln1_b into b1 (x1-hat feeds the FFN)
    W1_fold = (W1_f * g1[:, None]).astype(ml_dtypes.bfloat16)
    b1_fold = b1_f + be1 @ W1_f
    b1_r = np.ascontiguousarray(b1_fold.astype(np.float32).reshape(MFF, 128).T)
    W2_bf = np.ascontiguousarray(np.asarray(W2, np.float32).astype(ml_dtypes.bfloat16))
    common = dict(
        Wq=Wq_s,
        Wk=np.ascontiguousarray(np.asarray(Wk, np.float32).astype(ml_dtypes.bfloat16)),
        Wv=np.ascontiguousarray(np.asarray(Wv, np.float32).astype(ml_dtypes.bfloat16)),
        bq=bq_s, bk=bk_r, W1=np.ascontiguousarray(W1_fold), W2=W2_bf, b1=b1_r,
    )
    if not f_bv:
        bv_ext = np.zeros(H * HDE, ml_dtypes.bfloat16)
        bv_ext.reshape(H, HDE)[:, :HD] = bv_f.reshape(H, HD).astype(ml_dtypes.bfloat16)
        common["bv"] = bv_ext
    if not f_b2:
        common["b2"] = b2_f
    if not f_ln1:
        common["ln1g"] = g1
        common["ln1b"] = be1
    if not f_ln2:
        common["ln2g"] = g2
        common["ln2b"] = be2

    # combined band+edge masks [k_local, variant, head, kb*128+q]:
    # band: key block t keeps k>=q, block t+1 keeps k<=q
    kl = np.arange(128)[:, None]
    ql = np.arange(128)[None, :]
    band = np.zeros((128, 256), np.float32)
    band[:, 0:128] = (kl >= ql)
    band[:, 128:256] = (kl <= ql)

    in_maps = []
    for i in range(NCORES):
        bi, ci = divmod(i, S // TC)
        s0 = ci * TC
        xh = np.zeros((TH, DM), np.float32)
        lo, hi = max(0, s0 - WIN), min(S, s0 + TC + WIN)
        xh[lo - (s0 - WIN): hi - (s0 - WIN)] = xf[bi, lo:hi]
        mk = np.broadcast_to(band[:, None, None, :], (128, 3, 6, 256)).copy()
        if ci == 0:  # kg < 0 keys of block 0 (variant 1, used at t=0)
            mk[kl[:, 0] < WIN, 1, :, 0:128] = 0
        if ci == (S // TC) - 1:  # kg >= S keys of the last block (variant 2)
            mk[kl[:, 0] >= WIN, 2, :, 128:256] = 0
        in_maps.append(dict(
            xT=np.ascontiguousarray(xh.T.astype(ml_dtypes.bfloat16)),
            xres=np.ascontiguousarray(xf[bi, s0:s0 + TC]),
            mk=np.ascontiguousarray(mk.astype(ml_dtypes.bfloat16)), **common))
    return in_maps


def run_spmd(in_maps, trace=False):
    from concourse.bass_utils import run_bass_kernel_spmd
    from concourse import mybir
    flags = _FLAGS if _FLAGS is not None else (True, True, True, True)
    _get_program(flags)
    entry = _PROGS[tuple(flags)]
    if not entry["split"]:
        # walrus codegen limitation: <=1 sync wait per instruction
        _split_multi_waits(entry["nc"], mybir)
        entry["split"] = True
    return run_bass_kernel_spmd(entry["nc"], in_maps, list(range(NCORES)),
                                trace=trace)


def kernel(**inputs) -> np.ndarray:
    in_maps = make_in_maps(**inputs)
    res = run_spmd(in_maps).results
    outs = np.stack([np.asarray(res[i]["out"], np.float32) for i in range(NCORES)])
    return np.ascontiguousarray(outs.reshape(B, S, DM))
